# revision 1
# baseline (speedup 1.0000x reference)
"""Multi-head attention (B=8, N=1024, H=12, D=64, C=768) on 8 trn2 cores.

Sharding: data-parallel over batch. Core b computes attention for x[b];
weights are replicated. No collectives.

Per-core dataflow (all matmul operands float32r = full PE rate, fp32 bits):
  phase 1a: qkT[1536 x N] = W_qkv[:, :1536].T @ x^T    (d-major Q^T, K^T)
  phase 1b: v[N x 768]    = x @ W_qkv[:, 1536:]        (+ ones column per head)
  phase 2 (per head pair, heads 2t/2t+1 packed at partitions 0:64/64:128):
     S^T[m,n] = k^T.T @ q^T            (K=64 row-group packed pairs)
     P^T = exp(S^T / 8)                 (ScalarE, one [128,1024] op per m)
     outT[65,n] += v_aug[m].T @ P^T     (row 64 = rowsum via ones column)
     hT = outT[0:64] * bcast(1/rowsum)  (DVE mult; hT aliases the dead Q tile)
  phase 3: y = hT.T @ W_proj
"""
from contextlib import nullcontext

import numpy as np

import concourse.bass as bass
import concourse.mybir as mybir
import concourse.tile as tile
from concourse import bacc
from concourse.bass_utils import run_bass_kernel_spmd

F32R = mybir.dt.float32r
F32 = mybir.dt.float32

B, N, C = 8, 1024, 768
H, D = 12, 64
HID = H * D  # 768
KT = C // 128          # 6 feature k-tiles
MT = N // 128          # 8 sequence m-tiles
SCALE = D ** -0.5      # 0.125

_cached_nc = None

DEFAULT_OPTS = dict(
    s_bufs=2, acc_bufs=2, mm1_bufs=2, pt_bufs=4,
    eager_acc_evict=True, interleave_loads=True, proj_dual_pool=False,
    hoist_pair0=True,
)


def build_program(repeats=1, phases=("qk", "v", "attn", "proj"), **opts):
    o = dict(DEFAULT_OPTS, **opts)
    nc = bacc.Bacc(None, target_bir_lowering=False)

    xT_d = nc.dram_tensor("xT", [C, N], F32R, kind="ExternalInput")
    wqkv_d = nc.dram_tensor("wqkv", [C, 3 * HID], F32R, kind="ExternalInput")
    wproj_d = nc.dram_tensor("wproj", [HID, C], F32R, kind="ExternalInput")
    y_d = nc.dram_tensor("y", [N, C], F32, kind="ExternalOutput")

    with tile.TileContext(nc) as tc:
        with tc.tile_pool(name="persist", bufs=1) as persist, \
             tc.tile_pool(name="pt_pool", bufs=o["pt_bufs"]) as pt_pool, \
             tc.tile_pool(name="nrm_pool", bufs=3) as nrm_pool, \
             tc.tile_pool(name="y_pool", bufs=2) as y_pool, \
             tc.tile_pool(name="ps_a", bufs=o["mm1_bufs"], space="PSUM") as ps_a, \
             tc.tile_pool(name="ps_s", bufs=o["s_bufs"], space="PSUM") as ps_s, \
             tc.tile_pool(name="ps_acc", bufs=o["acc_bufs"], space="PSUM") as ps_acc:

            # ---- resident loads (emission order = DMA priority) ----
            xt = [persist.tile([128, N], F32R, name=f"xt{k}", tag=f"xt{k}")
                  for k in range(KT)]
            wqk = [persist.tile([128, 2 * HID], F32R, name=f"wqk{k}", tag=f"wqk{k}")
                   for k in range(KT)]
            wv = [persist.tile([128, HID], F32R, name=f"wv{k}", tag=f"wv{k}")
                  for k in range(KT)]
            if o["interleave_loads"]:
                for k in range(KT):
                    nc.sync.dma_start(xt[k][:], xT_d[k * 128:(k + 1) * 128, :])
                    nc.sync.dma_start(wqk[k][:],
                                      wqkv_d[k * 128:(k + 1) * 128, :2 * HID])
                for k in range(KT):
                    nc.sync.dma_start(wv[k][:], wqkv_d[k * 128:(k + 1) * 128, 2 * HID:])
            else:
                for k in range(KT):
                    nc.sync.dma_start(xt[k][:], xT_d[k * 128:(k + 1) * 128, :])
                for k in range(KT):
                    nc.sync.dma_start(wqk[k][:],
                                      wqkv_d[k * 128:(k + 1) * 128, :2 * HID])
                for k in range(KT):
                    nc.sync.dma_start(wv[k][:], wqkv_d[k * 128:(k + 1) * 128, 2 * HID:])

            # warm the exp table set during the DMA prefix (the ACT
            # table load otherwise lands on the first real exp)
            warm = persist.tile([1, 8], F32, name="warm", tag="warm")
            nc.gpsimd.memset(warm[:], 0.0)
            nc.scalar.activation(warm[:], warm[:],
                                 mybir.ActivationFunctionType.Exp)

            qkT = [persist.tile([128, N], F32R, name=f"qkT{t}", tag=f"qkT{t}")
                   for t in range(12)]
            v_aug = [persist.tile([128, H, D + 1], F32R, name=f"vaug{m}", tag=f"vaug{m}")
                     for m in range(MT)]
            hT = qkT[:6]  # normalized outputs overwrite the dead Q tiles

            # ---- phase 1a: one qkT tile (output rows = qkv cols t*128..) ----
            def qk_tile(t):
                for nh in range(2):
                    ps = ps_a.tile([128, 512], F32, name="ps_qk", tag="mm1")
                    for k in range(KT):
                        nc.tensor.matmul(ps[:], wqk[k][:, t * 128:(t + 1) * 128],
                                         xt[k][:, nh * 512:(nh + 1) * 512],
                                         start=(k == 0), stop=(k == KT - 1))
                    nc.vector.tensor_copy(qkT[t][:, nh * 512:(nh + 1) * 512], ps[:])

            # ---- phase 1b: v tiles ----
            def v_tile(m):
                for vh in range(2):
                    ps = ps_a.tile([128, 384], F32, name="ps_v", tag="mm1")
                    for k in range(KT):
                        nc.tensor.matmul(ps[:], xt[k][:, m * 128:(m + 1) * 128],
                                         wv[k][:, vh * 384:(vh + 1) * 384],
                                         start=(k == 0), stop=(k == KT - 1))
                    dst = v_aug[m][:, vh * 6:(vh + 1) * 6, 0:D]
                    nc.vector.tensor_copy(dst, ps[:].rearrange("p (h d) -> p h d", d=D))
                nc.gpsimd.memset(v_aug[m][:, :, D:D + 1].bitcast(F32), 1.0)

            # ---- phase 2: attention for head pair (2t, 2t+1) ----
            def attention(t, hoist=False):
                qT_t, kT_t = qkT[t], qkT[6 + t]
                for nh in range(2):
                    nsl = slice(nh * 512, (nh + 1) * 512)
                    acc = [ps_acc.tile([D + 1, 512], F32, name="acc", tag="acc")
                           for _ in range(2)]
                    for m in range(MT):
                        msl = slice(m * 128, (m + 1) * 128)
                        # both heads' S^T m-tile in one 2-bank psum tile;
                        # one [128,1024] exp serves both.
                        with tc.high_priority() if hoist else nullcontext():
                            s_ps = ps_s.tile([128, 1024], F32, name="s_ps", tag="s")
                            for j in range(2):
                                psl = slice(j * 64, (j + 1) * 64)
                                nc.tensor.matmul(s_ps[:, j * 512:(j + 1) * 512],
                                                 kT_t[psl, msl], qT_t[psl, nsl],
                                                 start=True, stop=True)
                            p_sb = pt_pool.tile([128, 1024], F32R, name="p_sb", tag="p")
                            nc.scalar.activation(p_sb[:], s_ps[:],
                                                 mybir.ActivationFunctionType.Exp,
                                                 scale=SCALE)
                        for j in range(2):
                            nc.tensor.matmul(acc[j][:], v_aug[m][:, 2 * t + j, :],
                                             p_sb[:, j * 512:(j + 1) * 512],
                                             start=(m == 0), stop=(m == MT - 1))
                    # normalize: rowsum sits in acc[j] row 64. HW
                    # partition_broadcast reads physical partition 0, so each
                    # reciprocal lives in its own [1, 512] tile.
                    for j in range(2):
                        rs = nrm_pool.tile([1, 512], F32, name="rs", tag="rs")
                        nc.vector.reciprocal(rs[0:1, :], acc[j][D:D + 1, :])
                        bc = nrm_pool.tile([64, 512], F32, name="bc", tag="bc")
                        nc.gpsimd.partition_broadcast(bc[:], rs[0:1, :])
                        if o["eager_acc_evict"]:
                            ev = pt_pool.tile([64, 512], F32, name="ev", tag="ev")
                            nc.vector.tensor_copy(ev[:], acc[j][0:D, :])
                            nc.vector.tensor_mul(hT[t][j * 64:(j + 1) * 64, nsl],
                                                 ev[:], bc[:])
                        else:
                            nc.vector.tensor_mul(hT[t][j * 64:(j + 1) * 64, nsl],
                                                 acc[j][0:D, :], bc[:])

            # ---- phase 3: y = hT.T @ W_proj ----
            def proj(m):
                for ph in range(2):
                    if o["proj_dual_pool"] and ph == 1:
                        ps = ps_s.tile([128, 384], F32, name="ps_y2", tag="s")
                    else:
                        ps = ps_a.tile([128, 384], F32, name="ps_y", tag="mm1")
                    for k in range(KT):
                        nc.tensor.matmul(ps[:], hT[k][:, m * 128:(m + 1) * 128],
                                         wp[k][:, ph * 384:(ph + 1) * 384],
                                         start=(k == 0), stop=(k == KT - 1))
                    y_sb = y_pool.tile([128, 384], F32, name="y_sb", tag="y")
                    if o.get("y_evict_dve"):
                        nc.vector.tensor_copy(y_sb[:], ps[:])
                    else:
                        nc.scalar.copy(y_sb[:], ps[:])
                    nc.sync.dma_start(
                        y_d[m * 128:(m + 1) * 128, ph * 384:(ph + 1) * 384], y_sb[:])

            for _ in range(repeats):
                if "qk" in phases:
                    qk_tile(0)
                    qk_tile(6)
                if "v" in phases:
                    for m in range(MT):
                        v_tile(m)
                if "qk" in phases and "attn" in phases:
                    # pair 0's S^T/exp get hoisted over the v-phase PE work
                    attention(0, hoist=o["hoist_pair0"])
                    for t in range(1, 6):
                        qk_tile(t)
                        qk_tile(6 + t)
                        attention(t)
                elif "qk" in phases:
                    for t in range(1, 6):
                        qk_tile(t)
                        qk_tile(6 + t)
                if "proj" in phases:
                    wp = [persist.tile([128, C], F32R, name=f"wp{k}", tag=f"wp{k}")
                          for k in range(KT)]
                    for k in range(KT):
                        nc.sync.dma_start(wp[k][:], wproj_d[k * 128:(k + 1) * 128, :])
                    for m in range(MT):
                        proj(m)

    nc.compile()
    return nc


def _run(inputs, trace=False, trace_kwargs=None):
    global _cached_nc
    x = np.asarray(inputs["x"], dtype=np.float32)
    wqkv = np.ascontiguousarray(np.asarray(inputs["W_qkv"], dtype=np.float32))
    wproj = np.ascontiguousarray(np.asarray(inputs["W_proj"], dtype=np.float32))
    xT = np.ascontiguousarray(x.transpose(0, 2, 1))  # [B, C, N]

    if _cached_nc is None:
        _cached_nc = build_program()
    nc = _cached_nc

    in_maps = [{"xT": xT[b], "wqkv": wqkv, "wproj": wproj} for b in range(B)]
    kwargs = {}
    if trace:
        kwargs["trace"] = True
        if trace_kwargs:
            kwargs.update(trace_kwargs)
    try:
        res = run_bass_kernel_spmd(nc, in_maps, core_ids=list(range(B)), **kwargs)
    except Exception:
        # transient axon/PJRT hiccups happen; one retry
        res = run_bass_kernel_spmd(nc, in_maps, core_ids=list(range(B)), **kwargs)
    out = np.stack([r["y"] for r in res.results], axis=0)
    return out, res


def kernel(**inputs):
    out, _ = _run(inputs)
    return out



# revision 30
# speedup vs baseline: 1.1552x; 1.1552x over previous
"""Multi-head attention (B=8, N=1024, H=12, D=64, C=768) on 8 trn2 cores.

Sharding: data-parallel over batch. Core b computes attention for x[b];
weights are replicated. No collectives.

v3 dataflow (fp16 matmul operands, fp32 PSUM accumulate):
  phase 1a: qkT[1536 x N] = W_qkv[:, :1536].T @ x^T    (d-major Q^T, K^T)
  phase 1b: v_aug[N x H x 65] = x @ W_qkv[:, 1536:]    (+ ones column)
  phase 2 (per head pair t=(2t,2t+1), per n-half nh, per m-tile):
     S^T[m,n] = k^T.T @ q^T                (2x K=64 matmuls, ap=512)
     P^T = exp(S^T / 8) -> fp16            (ScalarE, one [128,1024] op)
     acc[n,(j,nb),0:65] += P^T-block.T @ v_aug   (flipped PV: stationary
        = P^T [128m,128n] block, moving = v_aug [128m,65]; all 128
        n-partitions live -> 2.4x fewer PE cycles than v-stationary.
        col 64 accumulates the softmax denominator via the ones column.)
     normalize: h[n,d] = acc * (1/acc[:,:,64])  (DVE tensor_scalar)
     transpose: hT[d,n] via DMA xbar transpose (no PE/DVE cost)
  phase 3: y = hT.T @ W_proj
Pair-0 Q/K weight columns are DMA'd first (strided) so the exp stream
starts as soon as x has landed; x loads issue on the ACT HWDGE queue in
parallel with weight loads on the SP queue.
"""
from contextlib import nullcontext

import numpy as np

import concourse.bass as bass
import concourse.mybir as mybir
import concourse.tile as tile
from concourse import bacc
from concourse.bass_utils import run_bass_kernel_spmd
from concourse.masks import make_identity

F32 = mybir.dt.float32
F16 = mybir.dt.float16

B, N, C = 8, 1024, 768
H, D = 12, 64
HID = H * D  # 768
KT = C // 128          # 6 feature k-tiles
MT = N // 128          # 8 sequence m-tiles
SCALE = D ** -0.5      # 0.125

_cached_nc = None

DEFAULT_OPTS = dict(
    s_bufs=2, acc_bufs=1, mm1_bufs=2, pt_bufs=24, hoist_pair0=True,
)


def build_program(**opts):
    o = dict(DEFAULT_OPTS, **opts)
    nc = bacc.Bacc(None, target_bir_lowering=False)

    xT_d = nc.dram_tensor("xT", [C, N], F16, kind="ExternalInput")
    wqkv_d = nc.dram_tensor("wqkv", [C, 3 * HID], F16, kind="ExternalInput")
    wproj_d = nc.dram_tensor("wproj", [HID, C], F16, kind="ExternalInput")
    y_d = nc.dram_tensor("y", [N, C], F32, kind="ExternalOutput")

    with tile.TileContext(nc) as tc:
        with tc.tile_pool(name="persist", bufs=1) as persist, \
             tc.tile_pool(name="pt_pool", bufs=o["pt_bufs"]) as pt_pool, \
             tc.tile_pool(name="nrm_pool", bufs=6) as nrm_pool, \
             tc.tile_pool(name="y_pool", bufs=6) as y_pool, \
             tc.tile_pool(name="ps_a", bufs=o["mm1_bufs"], space="PSUM") as ps_a, \
             tc.tile_pool(name="ps_s", bufs=o["s_bufs"], space="PSUM") as ps_s, \
             tc.tile_pool(name="ps_acc", bufs=o["acc_bufs"], space="PSUM") as ps_acc:

            # ---- resident tiles (merged k-dim: fewer DMAs; the single
            # HWDGE device serializes descriptor generation at ~625ns per
            # DMA, so DMA count gates how fast inputs land) ----
            xt_t = persist.tile([128, KT, N], F16, name="xt", tag="xt")
            wqk06_t = persist.tile([128, 2, KT, 128], F16, name="wqk06",
                                   tag="wqk06")
            wqk_t = persist.tile([128, KT, 2 * HID], F16, name="wqk", tag="wqk")
            wv_t = persist.tile([128, KT, HID], F16, name="wv", tag="wv")
            wp_t = persist.tile([128, KT, C], F16, name="wp", tag="wp")
            xt = [xt_t[:, k, :] for k in range(KT)]
            wqk = [wqk_t[:, k, :] for k in range(KT)]
            wv = [wv_t[:, k, :] for k in range(KT)]
            wp = [wp_t[:, k, :] for k in range(KT)]

            # DMA priority: pair-0 qk weight cols + x first (feeds the
            # first two qk tiles and thus the exp stream), then v weights,
            # remaining qk weights, proj weights.
            xT_r = xT_d.rearrange("(k p) n -> p k n", p=128)
            wqkv_r = wqkv_d.rearrange("(k p) c -> p k c", p=128)
            wproj_r = wproj_d.rearrange("(k p) c -> p k c", p=128)
            nc.sync.dma_start(wqk06_t[:, 0], wqkv_r[:, :, 0:128])
            nc.sync.dma_start(wqk06_t[:, 1], wqkv_r[:, :, HID:HID + 128])
            for k in range(KT):
                nc.sync.dma_start(xt_t[:, k, :], xT_r[:, k, :])
            for i in range(2):
                nc.sync.dma_start(wv_t[:, 3 * i:3 * i + 3, :],
                                  wqkv_r[:, 3 * i:3 * i + 3, 2 * HID:])
            for i in range(3):
                nc.sync.dma_start(wqk_t[:, 2 * i:2 * i + 2, :],
                                  wqkv_r[:, 2 * i:2 * i + 2, :2 * HID])
            for i in range(2):
                nc.sync.dma_start(wp_t[:, 3 * i:3 * i + 3, :],
                                  wproj_r[:, 3 * i:3 * i + 3, :])

            # warm the exp table set during the DMA prefix (the ACT
            # table load otherwise lands on the first real exp)
            warm = persist.tile([1, 8], F32, name="warm", tag="warm")
            nc.gpsimd.memset(warm[:], 0.0)
            nc.scalar.activation(warm[:], warm[:],
                                 mybir.ActivationFunctionType.Exp)

            # PE p-state ramp: ~3us of back-to-back dummy matmuls while the
            # first DMAs land, so real matmuls start at max clock instead
            # of paying the 0.65/1.2 GHz warm-up on the critical path.
            junk = persist.tile([128, 128], F16, name="junk", tag="junk")
            nc.gpsimd.memset(junk[:], 0.0)
            ps_j = ps_a.tile([128, 128], F32, name="ps_junk", tag="mm1")
            for _ in range(30):
                nc.tensor.matmul(ps_j[:], junk[:], junk[:],
                                 start=True, stop=True)

            qkT = [persist.tile([128, N], F16, name=f"qkT{t}", tag=f"qkT{t}")
                   for t in range(12)]
            v_aug = [persist.tile([128, H, D + 1], F16, name=f"vaug{m}",
                                  tag=f"vaug{m}")
                     for m in range(MT)]
            hT = [persist.tile([128, N], F16, name=f"hT{t}", tag=f"hT{t}")
                  for t in range(KT)]

            # ---- phase 1a: one qkT tile (output rows = qkv cols t*128..) ----
            def qk_tile(t, halves=(0, 1)):
                for nhalf in halves:
                    ps = ps_a.tile([128, 512], F32, name="ps_qk", tag="mm1")
                    for k in range(KT):
                        if t == 0 or t == 6:
                            w = wqk06_t[:, 0 if t == 0 else 1, k, :]
                        else:
                            w = wqk[k][:, t * 128:(t + 1) * 128]
                        nc.tensor.matmul(ps[:], w,
                                         xt[k][:, nhalf * 512:(nhalf + 1) * 512],
                                         start=(k == 0), stop=(k == KT - 1))
                    nc.vector.tensor_copy(qkT[t][:, nhalf * 512:(nhalf + 1) * 512],
                                          ps[:])

            # ---- phase 1b: v tiles ----
            def v_tile(m):
                for vh in range(2):
                    ps = ps_a.tile([128, 384], F32, name="ps_v", tag="mm1")
                    for k in range(KT):
                        nc.tensor.matmul(ps[:], xt[k][:, m * 128:(m + 1) * 128],
                                         wv[k][:, vh * 384:(vh + 1) * 384],
                                         start=(k == 0), stop=(k == KT - 1))
                    dst = v_aug[m][:, vh * 6:(vh + 1) * 6, 0:D]
                    nc.vector.tensor_copy(dst,
                                          ps[:].rearrange("p (h d) -> p h d", d=D))
                nc.gpsimd.memset(v_aug[m][:, :, D:D + 1], 1.0)

            # Priority bands inside the attention stream: S+exp run at
            # absolute top priority (they feed ScalarE, the pacing engine);
            # normalize at ~5 and PVs at ~10 so a post-boundary PV backlog
            # can never delay the next S matmul; background (qk/v/proj)
            # keeps natural emission priorities (~100+).
            def band(prio):
                return tc.high_priority(offset=tc.cur_priority - prio)

            # ---- phase 2: attention for head pair (2t, 2t+1), n-half nh ----
            def attention_nh(t, nh, last=False):
                qT_t, kT_t = qkT[t], qkT[6 + t]
                nsl = slice(nh * 512, (nh + 1) * 512)
                acc = ps_acc.tile([128, 8, 128], F32, name="acc", tag="acc")
                for m in range(MT):
                    msl = slice(m * 128, (m + 1) * 128)
                    with tc.high_priority():
                        s_ps = ps_s.tile([128, 1024], F32, name="s_ps", tag="s")
                        for j in range(2):
                            psl = slice(j * 64, (j + 1) * 64)
                            nc.tensor.matmul(s_ps[:, j * 512:(j + 1) * 512],
                                             kT_t[psl, msl], qT_t[psl, nsl],
                                             start=True, stop=True)
                        p_sb = pt_pool.tile([128, 1024], F16, name="p_sb", tag="p")
                        nc.scalar.activation(p_sb[:], s_ps[:],
                                             mybir.ActivationFunctionType.Exp,
                                             scale=SCALE)
                    with band(10):
                        for j in range(2):
                            for nb in range(4):
                                nc.tensor.matmul(
                                    acc[:, j * 4 + nb, 0:D + 1],
                                    p_sb[:, j * 512 + nb * 128:j * 512 + (nb + 1) * 128],
                                    v_aug[m][:, 2 * t + j, :],
                                    start=(m == 0 and nb == 0),
                                    stop=(m == MT - 1))
                # normalize (DVE per-partition scalar), then transpose into
                # hT: DMA xbar transpose (no PE/DVE cost) except for the
                # last group, where PE-transpose latency is lower and the
                # final proj tiles are gated on it.
                with band(5):
                    rs = nrm_pool.tile([128, 8], F32, name="rs", tag="rs")
                    nc.vector.reciprocal(rs[:], acc[:, :, D])
                    for nb in range(4):
                        hst = nrm_pool.tile([128, 128], F16, name="hst", tag="hst")
                        for j in range(2):
                            nc.vector.tensor_scalar_mul(
                                hst[:, j * D:(j + 1) * D],
                                acc[:, j * 4 + nb, 0:D],
                                rs[:, j * 4 + nb:j * 4 + nb + 1])
                        nc.sync.dma_start_transpose(
                            hT[t][:, nh * 512 + nb * 128:nh * 512 + (nb + 1) * 128],
                            hst[:])

            # ---- phase 3: y = hT.T @ W_proj ----
            def proj(m, direct=False):
                for ph in range(2):
                    ps = ps_a.tile([128, 384], F32, name="ps_y", tag="mm1")
                    for k in range(KT):
                        nc.tensor.matmul(ps[:], hT[k][:, m * 128:(m + 1) * 128],
                                         wp[k][:, ph * 384:(ph + 1) * 384],
                                         start=(k == 0), stop=(k == KT - 1))
                    ysl = (slice(m * 128, (m + 1) * 128),
                           slice(ph * 384, (ph + 1) * 384))
                    yq = nc.sync if (2 * m + ph) % 2 == 0 else nc.scalar
                    if direct:
                        # final tiles: DMA straight from PSUM, skipping the
                        # staging hop (nothing needs the buffer afterwards)
                        yq.dma_start(y_d[ysl[0], ysl[1]], ps[:])
                    else:
                        y_sb = y_pool.tile([128, 384], F32, name="y_sb", tag="y")
                        nc.vector.tensor_copy(y_sb[:], ps[:])
                        yq.dma_start(y_d[ysl[0], ysl[1]], y_sb[:])

            # ---- schedule ----
            # Emission order = dataflow order (tile deps derive from it);
            # the scheduler's greedy choice is steered by the bands above.
            qk_tile(0, halves=(0,))
            qk_tile(6, halves=(0,))
            qk_tile(0, halves=(1,))
            qk_tile(6, halves=(1,))
            qk_tile(1)
            qk_tile(7)
            for m in range(MT):
                v_tile(m)
            attention_nh(0, 0)
            for t in range(1, 6):
                if t < 5:
                    qk_tile(t + 1)
                    qk_tile(6 + t + 1)
                attention_nh(t, 0)
            for m in range(4):
                proj(m)
            for t in range(6):
                attention_nh(t, 1)
            for m in range(4, MT):
                proj(m)

    nc.compile()
    return nc


def _run(inputs, trace=False, trace_kwargs=None):
    global _cached_nc
    x = np.asarray(inputs["x"], dtype=np.float32)
    wqkv = np.ascontiguousarray(
        np.asarray(inputs["W_qkv"], dtype=np.float32)).astype(np.float16)
    wproj = np.ascontiguousarray(
        np.asarray(inputs["W_proj"], dtype=np.float32)).astype(np.float16)
    xT = np.ascontiguousarray(x.transpose(0, 2, 1)).astype(np.float16)

    if _cached_nc is None:
        _cached_nc = build_program()
    nc = _cached_nc

    in_maps = [{"xT": xT[b], "wqkv": wqkv, "wproj": wproj} for b in range(B)]
    kwargs = {}
    if trace:
        kwargs["trace"] = True
        if trace_kwargs:
            kwargs.update(trace_kwargs)
    try:
        res = run_bass_kernel_spmd(nc, in_maps, core_ids=list(range(B)), **kwargs)
    except Exception:
        # transient axon/PJRT hiccups happen; one retry
        res = run_bass_kernel_spmd(nc, in_maps, core_ids=list(range(B)), **kwargs)
    out = np.stack([r["y"] for r in res.results], axis=0)
    return out, res


def kernel(**inputs):
    out, _ = _run(inputs)
    return out


# revision 32
# speedup vs baseline: 1.2030x; 1.0414x over previous
"""Multi-head attention (B=8, N=1024, H=12, D=64, C=768) on 8 trn2 cores.

Sharding: data-parallel over batch. Core b computes attention for x[b];
weights are replicated. No collectives.

v3 dataflow (fp16 matmul operands, fp32 PSUM accumulate):
  phase 1a: qkT[1536 x N] = W_qkv[:, :1536].T @ x^T    (d-major Q^T, K^T)
  phase 1b: v_aug[N x H x 65] = x @ W_qkv[:, 1536:]    (+ ones column)
  phase 2 (per head pair t=(2t,2t+1), per n-half nh, per m-tile):
     S^T[m,n] = k^T.T @ q^T                (2x K=64 matmuls, ap=512)
     P^T = exp(S^T / 8) -> fp16            (ScalarE, one [128,1024] op)
     acc[n,(j,nb),0:65] += P^T-block.T @ v_aug   (flipped PV: stationary
        = P^T [128m,128n] block, moving = v_aug [128m,65]; all 128
        n-partitions live -> 2.4x fewer PE cycles than v-stationary.
        col 64 accumulates the softmax denominator via the ones column.)
     normalize: h[n,d] = acc * (1/acc[:,:,64])  (DVE tensor_scalar)
     transpose: hT[d,n] via DMA xbar transpose (no PE/DVE cost)
  phase 3: y = hT.T @ W_proj
Pair-0 Q/K weight columns are DMA'd first (strided) so the exp stream
starts as soon as x has landed; x loads issue on the ACT HWDGE queue in
parallel with weight loads on the SP queue.
"""
from contextlib import nullcontext

import numpy as np

import concourse.bass as bass
import concourse.mybir as mybir
import concourse.tile as tile
from concourse import bacc
from concourse.bass_utils import run_bass_kernel_spmd
from concourse.masks import make_identity

F32 = mybir.dt.float32
F16 = mybir.dt.float16

B, N, C = 8, 1024, 768
H, D = 12, 64
HID = H * D  # 768
KT = C // 128          # 6 feature k-tiles
MT = N // 128          # 8 sequence m-tiles
SCALE = D ** -0.5      # 0.125

_cached_nc = None

DEFAULT_OPTS = dict(
    s_bufs=2, acc_bufs=1, mm1_bufs=2, pt_bufs=24, hoist_pair0=True,
)


def build_program(**opts):
    o = dict(DEFAULT_OPTS, **opts)
    nc = bacc.Bacc(None, target_bir_lowering=False)

    xT_d = nc.dram_tensor("xT", [C, N], F16, kind="ExternalInput")
    wqkv_d = nc.dram_tensor("wqkv", [C, 3 * HID], F16, kind="ExternalInput")
    wproj_d = nc.dram_tensor("wproj", [HID, C], F16, kind="ExternalInput")
    y_d = nc.dram_tensor("y", [N, C], F32, kind="ExternalOutput")

    with tile.TileContext(nc) as tc:
        with tc.tile_pool(name="persist", bufs=1) as persist, \
             tc.tile_pool(name="pt_pool", bufs=o["pt_bufs"]) as pt_pool, \
             tc.tile_pool(name="nrm_pool", bufs=6) as nrm_pool, \
             tc.tile_pool(name="y_pool", bufs=6) as y_pool, \
             tc.tile_pool(name="ps_a", bufs=o["mm1_bufs"], space="PSUM") as ps_a, \
             tc.tile_pool(name="ps_s", bufs=o["s_bufs"], space="PSUM") as ps_s, \
             tc.tile_pool(name="ps_acc", bufs=o["acc_bufs"], space="PSUM") as ps_acc:

            # ---- resident tiles (merged k-dim: fewer DMAs; the single
            # HWDGE device serializes descriptor generation at ~625ns per
            # DMA, so DMA count gates how fast inputs land) ----
            xt_t = persist.tile([128, KT, N], F16, name="xt", tag="xt")
            wqk06_t = persist.tile([128, 2, KT, 128], F16, name="wqk06",
                                   tag="wqk06")
            wqk_t = persist.tile([128, KT, 2 * HID], F16, name="wqk", tag="wqk")
            wv_t = persist.tile([128, KT, HID], F16, name="wv", tag="wv")
            wp_t = persist.tile([128, KT, C], F16, name="wp", tag="wp")
            xt = [xt_t[:, k, :] for k in range(KT)]
            wqk = [wqk_t[:, k, :] for k in range(KT)]
            wv = [wv_t[:, k, :] for k in range(KT)]
            wp = [wp_t[:, k, :] for k in range(KT)]

            # DMA priority: pair-0 qk weight cols + x first (feeds the
            # first two qk tiles and thus the exp stream), then v weights,
            # remaining qk weights, proj weights.
            xT_r = xT_d.rearrange("(k p) n -> p k n", p=128)
            wqkv_r = wqkv_d.rearrange("(k p) c -> p k c", p=128)
            wproj_r = wproj_d.rearrange("(k p) c -> p k c", p=128)
            nc.sync.dma_start(wqk06_t[:, 0], wqkv_r[:, :, 0:128])
            nc.sync.dma_start(wqk06_t[:, 1], wqkv_r[:, :, HID:HID + 128])
            for k in range(KT):
                nc.sync.dma_start(xt_t[:, k, :], xT_r[:, k, :])
            for i in range(2):
                nc.sync.dma_start(wv_t[:, 3 * i:3 * i + 3, :],
                                  wqkv_r[:, 3 * i:3 * i + 3, 2 * HID:])
            for i in range(3):
                nc.sync.dma_start(wqk_t[:, 2 * i:2 * i + 2, :],
                                  wqkv_r[:, 2 * i:2 * i + 2, :2 * HID])
            for i in range(2):
                nc.sync.dma_start(wp_t[:, 3 * i:3 * i + 3, :],
                                  wproj_r[:, 3 * i:3 * i + 3, :])

            # warm the exp table set during the DMA prefix (the ACT
            # table load otherwise lands on the first real exp)
            warm = persist.tile([1, 8], F32, name="warm", tag="warm")
            nc.gpsimd.memset(warm[:], 0.0)
            nc.scalar.activation(warm[:], warm[:],
                                 mybir.ActivationFunctionType.Exp)

            # identity for the last-group PE transpose
            ident = persist.tile([128, 128], F16, name="ident", tag="ident")
            make_identity(nc, ident[:])

            # PE p-state ramp: ~3us of back-to-back dummy matmuls while the
            # first DMAs land, so real matmuls start at max clock instead
            # of paying the 0.65/1.2 GHz warm-up on the critical path.
            junk = persist.tile([128, 128], F16, name="junk", tag="junk")
            nc.gpsimd.memset(junk[:], 0.0)
            ps_j = ps_a.tile([128, 128], F32, name="ps_junk", tag="mm1")
            for _ in range(30):
                nc.tensor.matmul(ps_j[:], junk[:], junk[:],
                                 start=True, stop=True)

            qkT = [persist.tile([128, N], F16, name=f"qkT{t}", tag=f"qkT{t}")
                   for t in range(12)]
            v_aug = [persist.tile([128, H, D + 1], F16, name=f"vaug{m}",
                                  tag=f"vaug{m}")
                     for m in range(MT)]
            hT = [persist.tile([128, N], F16, name=f"hT{t}", tag=f"hT{t}")
                  for t in range(KT)]

            # ---- phase 1a: one qkT tile (output rows = qkv cols t*128..) ----
            def qk_tile(t, halves=(0, 1)):
                for nhalf in halves:
                    ps = ps_a.tile([128, 512], F32, name="ps_qk", tag="mm1")
                    for k in range(KT):
                        if t == 0 or t == 6:
                            w = wqk06_t[:, 0 if t == 0 else 1, k, :]
                        else:
                            w = wqk[k][:, t * 128:(t + 1) * 128]
                        nc.tensor.matmul(ps[:], w,
                                         xt[k][:, nhalf * 512:(nhalf + 1) * 512],
                                         start=(k == 0), stop=(k == KT - 1))
                    nc.vector.tensor_copy(qkT[t][:, nhalf * 512:(nhalf + 1) * 512],
                                          ps[:])

            # ---- phase 1b: v tiles ----
            def v_tile(m):
                for vh in range(2):
                    ps = ps_a.tile([128, 384], F32, name="ps_v", tag="mm1")
                    for k in range(KT):
                        nc.tensor.matmul(ps[:], xt[k][:, m * 128:(m + 1) * 128],
                                         wv[k][:, vh * 384:(vh + 1) * 384],
                                         start=(k == 0), stop=(k == KT - 1))
                    dst = v_aug[m][:, vh * 6:(vh + 1) * 6, 0:D]
                    nc.vector.tensor_copy(dst,
                                          ps[:].rearrange("p (h d) -> p h d", d=D))
                nc.gpsimd.memset(v_aug[m][:, :, D:D + 1], 1.0)

            # Priority bands inside the attention stream: S+exp run at
            # absolute top priority (they feed ScalarE, the pacing engine);
            # normalize at ~5 and PVs at ~10 so a post-boundary PV backlog
            # can never delay the next S matmul; background (qk/v/proj)
            # keeps natural emission priorities (~100+).
            def band(prio):
                return tc.high_priority(offset=tc.cur_priority - prio)

            # ---- phase 2: attention for head pair (2t, 2t+1), n-half nh ----
            def attention_nh(t, nh, last=False):
                qT_t, kT_t = qkT[t], qkT[6 + t]
                nsl = slice(nh * 512, (nh + 1) * 512)
                acc = ps_acc.tile([128, 8, 128], F32, name="acc", tag="acc")
                for m in range(MT):
                    msl = slice(m * 128, (m + 1) * 128)
                    with tc.high_priority():
                        s_ps = ps_s.tile([128, 1024], F32, name="s_ps", tag="s")
                        for j in range(2):
                            psl = slice(j * 64, (j + 1) * 64)
                            nc.tensor.matmul(s_ps[:, j * 512:(j + 1) * 512],
                                             kT_t[psl, msl], qT_t[psl, nsl],
                                             start=True, stop=True)
                        p_sb = pt_pool.tile([128, 1024], F16, name="p_sb", tag="p")
                        nc.scalar.activation(p_sb[:], s_ps[:],
                                             mybir.ActivationFunctionType.Exp,
                                             scale=SCALE)
                    with band(10):
                        for j in range(2):
                            for nb in range(4):
                                nc.tensor.matmul(
                                    acc[:, j * 4 + nb, 0:D + 1],
                                    p_sb[:, j * 512 + nb * 128:j * 512 + (nb + 1) * 128],
                                    v_aug[m][:, 2 * t + j, :],
                                    start=(m == 0 and nb == 0),
                                    stop=(m == MT - 1))
                # normalize (DVE per-partition scalar), then transpose into
                # hT: DMA xbar transpose (no PE/DVE cost) except for the
                # last group, where PE-transpose latency is lower and the
                # final proj tiles are gated on it.
                with band(5):
                    rs = nrm_pool.tile([128, 8], F32, name="rs", tag="rs")
                    nc.vector.reciprocal(rs[:], acc[:, :, D])
                    for nb in range(4):
                        hst = nrm_pool.tile([128, 128], F16, name="hst", tag="hst")
                        for j in range(2):
                            nc.vector.tensor_scalar_mul(
                                hst[:, j * D:(j + 1) * D],
                                acc[:, j * 4 + nb, 0:D],
                                rs[:, j * 4 + nb:j * 4 + nb + 1])
                        csl = slice(nh * 512 + nb * 128, nh * 512 + (nb + 1) * 128)
                        if last:
                            # PE transpose: lower latency than the DMA xbar
                            # path; each transpose gets its own psum tile so
                            # its start=True bank-zeroing touches nothing
                            # else (a start wipes the whole 2KB bank row).
                            tp = ps_a.tile([128, 128], F16, name="tp", tag="mm1")
                            nc.tensor.transpose(tp[:], hst[:], ident[:])
                            nc.scalar.copy(hT[t][:, csl], tp[:])
                        else:
                            nc.sync.dma_start_transpose(hT[t][:, csl], hst[:])

            # ---- phase 3: y = hT.T @ W_proj, split by k so most of the
            # work unlocks after head-pair 3 and fills PE slack; only the
            # k=4..5 remainder is gated on the last pairs ----
            y_part = [persist.tile([128, 2, 384], F32, name=f"yp{m}",
                                   tag=f"yp{m}")
                      for m in range(MT)]

            def proj_pass1(m):
                for ph in range(2):
                    ps = ps_a.tile([128, 384], F32, name="ps_y1", tag="mm1")
                    for k in range(4):
                        nc.tensor.matmul(ps[:], hT[k][:, m * 128:(m + 1) * 128],
                                         wp[k][:, ph * 384:(ph + 1) * 384],
                                         start=(k == 0), stop=(k == 3))
                    nc.vector.tensor_copy(y_part[m][:, ph, :], ps[:])

            def proj_pass2(m):
                for ph in range(2):
                    ps = ps_a.tile([128, 384], F32, name="ps_y2", tag="mm1")
                    for k in range(4, KT):
                        nc.tensor.matmul(ps[:], hT[k][:, m * 128:(m + 1) * 128],
                                         wp[k][:, ph * 384:(ph + 1) * 384],
                                         start=(k == 4), stop=(k == KT - 1))
                    y_sb = y_pool.tile([128, 384], F32, name="y_sb", tag="y")
                    nc.vector.tensor_tensor(y_sb[:], ps[:], y_part[m][:, ph, :],
                                            mybir.AluOpType.add)
                    yq = nc.sync if (2 * m + ph) % 2 == 0 else nc.scalar
                    yq.dma_start(
                        y_d[m * 128:(m + 1) * 128, ph * 384:(ph + 1) * 384],
                        y_sb[:])

            # ---- schedule ----
            # Emission order = dataflow order (tile deps derive from it);
            # the scheduler's greedy choice is steered by the bands above.
            qk_tile(0, halves=(0,))
            qk_tile(6, halves=(0,))
            qk_tile(0, halves=(1,))
            qk_tile(6, halves=(1,))
            qk_tile(1)
            qk_tile(7)
            for m in range(MT):
                v_tile(m)
            attention_nh(0, 0)
            for t in range(1, 6):
                if t < 5:
                    qk_tile(t + 1)
                    qk_tile(6 + t + 1)
                attention_nh(t, 0)
                if t == 3:
                    for m in range(4):
                        proj_pass1(m)
            for m in range(4):
                proj_pass2(m)
            for t in range(6):
                attention_nh(t, 1, last=(t == 5))
                if t == 3:
                    for m in range(4, MT):
                        proj_pass1(m)
            for m in range(4, MT):
                proj_pass2(m)

    nc.compile()
    return nc


def _run(inputs, trace=False, trace_kwargs=None):
    global _cached_nc
    x = np.asarray(inputs["x"], dtype=np.float32)
    wqkv = np.ascontiguousarray(
        np.asarray(inputs["W_qkv"], dtype=np.float32)).astype(np.float16)
    wproj = np.ascontiguousarray(
        np.asarray(inputs["W_proj"], dtype=np.float32)).astype(np.float16)
    xT = np.ascontiguousarray(x.transpose(0, 2, 1)).astype(np.float16)

    if _cached_nc is None:
        _cached_nc = build_program()
    nc = _cached_nc

    in_maps = [{"xT": xT[b], "wqkv": wqkv, "wproj": wproj} for b in range(B)]
    kwargs = {}
    if trace:
        kwargs["trace"] = True
        if trace_kwargs:
            kwargs.update(trace_kwargs)
    try:
        res = run_bass_kernel_spmd(nc, in_maps, core_ids=list(range(B)), **kwargs)
    except Exception:
        # transient axon/PJRT hiccups happen; one retry
        res = run_bass_kernel_spmd(nc, in_maps, core_ids=list(range(B)), **kwargs)
    out = np.stack([r["y"] for r in res.results], axis=0)
    return out, res


def kernel(**inputs):
    out, _ = _run(inputs)
    return out


# revision 34
# speedup vs baseline: 1.2053x; 1.0019x over previous
"""Multi-head attention (B=8, N=1024, H=12, D=64, C=768) on 8 trn2 cores.

Sharding: data-parallel over batch. Core b computes attention for x[b];
weights are replicated. No collectives.

v3 dataflow (fp16 matmul operands, fp32 PSUM accumulate):
  phase 1a: qkT[1536 x N] = W_qkv[:, :1536].T @ x^T    (d-major Q^T, K^T)
  phase 1b: v_aug[N x H x 65] = x @ W_qkv[:, 1536:]    (+ ones column)
  phase 2 (per head pair t=(2t,2t+1), per n-half nh, per m-tile):
     S^T[m,n] = k^T.T @ q^T                (2x K=64 matmuls, ap=512)
     P^T = exp(S^T / 8) -> fp16            (ScalarE, one [128,1024] op)
     acc[n,(j,nb),0:65] += P^T-block.T @ v_aug   (flipped PV: stationary
        = P^T [128m,128n] block, moving = v_aug [128m,65]; all 128
        n-partitions live -> 2.4x fewer PE cycles than v-stationary.
        col 64 accumulates the softmax denominator via the ones column.)
     normalize: h[n,d] = acc * (1/acc[:,:,64])  (DVE tensor_scalar)
     transpose: hT[d,n] via DMA xbar transpose (no PE/DVE cost)
  phase 3: y = hT.T @ W_proj
Pair-0 Q/K weight columns are DMA'd first (strided) so the exp stream
starts as soon as x has landed; x loads issue on the ACT HWDGE queue in
parallel with weight loads on the SP queue.
"""
from contextlib import nullcontext

import numpy as np

import concourse.bass as bass
import concourse.mybir as mybir
import concourse.tile as tile
from concourse import bacc
from concourse.bass_utils import run_bass_kernel_spmd
from concourse.masks import make_identity

F32 = mybir.dt.float32
F16 = mybir.dt.float16

B, N, C = 8, 1024, 768
H, D = 12, 64
HID = H * D  # 768
KT = C // 128          # 6 feature k-tiles
MT = N // 128          # 8 sequence m-tiles
SCALE = D ** -0.5      # 0.125

_cached_nc = None

DEFAULT_OPTS = dict(
    s_bufs=2, acc_bufs=1, mm1_bufs=2, pt_bufs=28, hoist_pair0=True,
)


def build_program(**opts):
    o = dict(DEFAULT_OPTS, **opts)
    nc = bacc.Bacc(None, target_bir_lowering=False)

    xT_d = nc.dram_tensor("xT", [C, N], F16, kind="ExternalInput")
    wqkv_d = nc.dram_tensor("wqkv", [C, 3 * HID], F16, kind="ExternalInput")
    wproj_d = nc.dram_tensor("wproj", [HID, C], F16, kind="ExternalInput")
    y_d = nc.dram_tensor("y", [N, C], F32, kind="ExternalOutput")

    with tile.TileContext(nc) as tc:
        with tc.tile_pool(name="persist", bufs=1) as persist, \
             tc.tile_pool(name="pt_pool", bufs=o["pt_bufs"]) as pt_pool, \
             tc.tile_pool(name="nrm_pool", bufs=6) as nrm_pool, \
             tc.tile_pool(name="y_pool", bufs=6) as y_pool, \
             tc.tile_pool(name="ps_a", bufs=o["mm1_bufs"], space="PSUM") as ps_a, \
             tc.tile_pool(name="ps_s", bufs=o["s_bufs"], space="PSUM") as ps_s, \
             tc.tile_pool(name="ps_acc", bufs=o["acc_bufs"], space="PSUM") as ps_acc:

            # ---- resident tiles (merged k-dim: fewer DMAs; the single
            # HWDGE device serializes descriptor generation at ~625ns per
            # DMA, so DMA count gates how fast inputs land) ----
            xt_t = persist.tile([128, KT, N], F16, name="xt", tag="xt")
            wqk06_t = persist.tile([128, 2, KT, 128], F16, name="wqk06",
                                   tag="wqk06")
            wqk_t = persist.tile([128, KT, 2 * HID], F16, name="wqk", tag="wqk")
            wv_t = persist.tile([128, KT, HID], F16, name="wv", tag="wv")
            wp_t = persist.tile([128, KT, C], F16, name="wp", tag="wp")
            xt = [xt_t[:, k, :] for k in range(KT)]
            wqk = [wqk_t[:, k, :] for k in range(KT)]
            wv = [wv_t[:, k, :] for k in range(KT)]
            wp = [wp_t[:, k, :] for k in range(KT)]

            # DMA priority: pair-0 qk weight cols + x first (feeds the
            # first two qk tiles and thus the exp stream), then v weights,
            # remaining qk weights, proj weights.
            xT_r = xT_d.rearrange("(k p) n -> p k n", p=128)
            wqkv_r = wqkv_d.rearrange("(k p) c -> p k c", p=128)
            wproj_r = wproj_d.rearrange("(k p) c -> p k c", p=128)
            nc.sync.dma_start(wqk06_t[:, 0], wqkv_r[:, :, 0:128])
            nc.sync.dma_start(xt_t[:, 0, :], xT_r[:, 0, :])
            nc.sync.dma_start(wqk06_t[:, 1], wqkv_r[:, :, HID:HID + 128])
            for k in range(1, KT):
                nc.sync.dma_start(xt_t[:, k, :], xT_r[:, k, :])
            for i in range(2):
                nc.sync.dma_start(wv_t[:, 3 * i:3 * i + 3, :],
                                  wqkv_r[:, 3 * i:3 * i + 3, 2 * HID:])
            for i in range(3):
                nc.sync.dma_start(wqk_t[:, 2 * i:2 * i + 2, :],
                                  wqkv_r[:, 2 * i:2 * i + 2, :2 * HID])
            for i in range(2):
                nc.sync.dma_start(wp_t[:, 3 * i:3 * i + 3, :],
                                  wproj_r[:, 3 * i:3 * i + 3, :])

            # warm the exp table set during the DMA prefix (the ACT
            # table load otherwise lands on the first real exp)
            warm = persist.tile([1, 8], F32, name="warm", tag="warm")
            nc.gpsimd.memset(warm[:], 0.0)
            nc.scalar.activation(warm[:], warm[:],
                                 mybir.ActivationFunctionType.Exp)

            # identity for the last-group PE transpose
            ident = persist.tile([128, 128], F16, name="ident", tag="ident")
            make_identity(nc, ident[:])

            # PE p-state ramp: ~3us of back-to-back dummy matmuls while the
            # first DMAs land, so real matmuls start at max clock instead
            # of paying the 0.65/1.2 GHz warm-up on the critical path.
            junk = persist.tile([128, 128], F16, name="junk", tag="junk")
            nc.gpsimd.memset(junk[:], 0.0)
            ps_j = ps_a.tile([128, 128], F32, name="ps_junk", tag="mm1")
            for _ in range(40):
                nc.tensor.matmul(ps_j[:], junk[:], junk[:],
                                 start=True, stop=True)

            qkT = [persist.tile([128, N], F16, name=f"qkT{t}", tag=f"qkT{t}")
                   for t in range(12)]
            v_aug = [persist.tile([128, H, D + 1], F16, name=f"vaug{m}",
                                  tag=f"vaug{m}")
                     for m in range(MT)]
            hT = [persist.tile([128, N], F16, name=f"hT{t}", tag=f"hT{t}")
                  for t in range(KT)]

            # ---- phase 1a: one qkT tile (output rows = qkv cols t*128..) ----
            def qk_tile(t, halves=(0, 1)):
                for nhalf in halves:
                    ps = ps_a.tile([128, 512], F32, name="ps_qk", tag="mm1")
                    for k in range(KT):
                        if t == 0 or t == 6:
                            w = wqk06_t[:, 0 if t == 0 else 1, k, :]
                        else:
                            w = wqk[k][:, t * 128:(t + 1) * 128]
                        nc.tensor.matmul(ps[:], w,
                                         xt[k][:, nhalf * 512:(nhalf + 1) * 512],
                                         start=(k == 0), stop=(k == KT - 1))
                    nc.vector.tensor_copy(qkT[t][:, nhalf * 512:(nhalf + 1) * 512],
                                          ps[:])

            # ---- phase 1b: v tiles ----
            def v_tile(m):
                for vh in range(2):
                    ps = ps_a.tile([128, 384], F32, name="ps_v", tag="mm1")
                    for k in range(KT):
                        nc.tensor.matmul(ps[:], xt[k][:, m * 128:(m + 1) * 128],
                                         wv[k][:, vh * 384:(vh + 1) * 384],
                                         start=(k == 0), stop=(k == KT - 1))
                    dst = v_aug[m][:, vh * 6:(vh + 1) * 6, 0:D]
                    nc.vector.tensor_copy(dst,
                                          ps[:].rearrange("p (h d) -> p h d", d=D))
                nc.gpsimd.memset(v_aug[m][:, :, D:D + 1], 1.0)

            # Priority bands inside the attention stream: S+exp run at
            # absolute top priority (they feed ScalarE, the pacing engine);
            # normalize at ~5 and PVs at ~10 so a post-boundary PV backlog
            # can never delay the next S matmul; background (qk/v/proj)
            # keeps natural emission priorities (~100+).
            def band(prio):
                return tc.high_priority(offset=tc.cur_priority - prio)

            # ---- phase 2: attention for head pair (2t, 2t+1), n-half nh ----
            def attention_nh(t, nh, last=False):
                qT_t, kT_t = qkT[t], qkT[6 + t]
                nsl = slice(nh * 512, (nh + 1) * 512)
                acc = ps_acc.tile([128, 8, 128], F32, name="acc", tag="acc")
                for m in range(MT):
                    msl = slice(m * 128, (m + 1) * 128)
                    with tc.high_priority():
                        s_ps = ps_s.tile([128, 1024], F32, name="s_ps", tag="s")
                        for j in range(2):
                            psl = slice(j * 64, (j + 1) * 64)
                            nc.tensor.matmul(s_ps[:, j * 512:(j + 1) * 512],
                                             kT_t[psl, msl], qT_t[psl, nsl],
                                             start=True, stop=True)
                        p_sb = pt_pool.tile([128, 1024], F16, name="p_sb", tag="p")
                        nc.scalar.activation(p_sb[:], s_ps[:],
                                             mybir.ActivationFunctionType.Exp,
                                             scale=SCALE)
                    with band(10):
                        for j in range(2):
                            for nb in range(4):
                                nc.tensor.matmul(
                                    acc[:, j * 4 + nb, 0:D + 1],
                                    p_sb[:, j * 512 + nb * 128:j * 512 + (nb + 1) * 128],
                                    v_aug[m][:, 2 * t + j, :],
                                    start=(m == 0 and nb == 0),
                                    stop=(m == MT - 1))
                # normalize (DVE per-partition scalar), then transpose into
                # hT: DMA xbar transpose (no PE/DVE cost) except for the
                # last group, where PE-transpose latency is lower and the
                # final proj tiles are gated on it.
                with band(5):
                    rs = nrm_pool.tile([128, 8], F32, name="rs", tag="rs")
                    nc.vector.reciprocal(rs[:], acc[:, :, D])
                    for nb in range(4):
                        hst = nrm_pool.tile([128, 128], F16, name="hst", tag="hst")
                        for j in range(2):
                            nc.vector.tensor_scalar_mul(
                                hst[:, j * D:(j + 1) * D],
                                acc[:, j * 4 + nb, 0:D],
                                rs[:, j * 4 + nb:j * 4 + nb + 1])
                        csl = slice(nh * 512 + nb * 128, nh * 512 + (nb + 1) * 128)
                        if last:
                            # PE transpose: lower latency than the DMA xbar
                            # path; each transpose gets its own psum tile so
                            # its start=True bank-zeroing touches nothing
                            # else (a start wipes the whole 2KB bank row).
                            tp = ps_a.tile([128, 128], F16, name="tp", tag="mm1")
                            nc.tensor.transpose(tp[:], hst[:], ident[:])
                            nc.scalar.copy(hT[t][:, csl], tp[:])
                        else:
                            nc.sync.dma_start_transpose(hT[t][:, csl], hst[:])

            # ---- phase 3: y = hT.T @ W_proj ----
            def proj(m):
                for ph in range(2):
                    ps = ps_a.tile([128, 384], F32, name="ps_y", tag="mm1")
                    for k in range(KT):
                        nc.tensor.matmul(ps[:], hT[k][:, m * 128:(m + 1) * 128],
                                         wp[k][:, ph * 384:(ph + 1) * 384],
                                         start=(k == 0), stop=(k == KT - 1))
                    y_sb = y_pool.tile([128, 384], F32, name="y_sb", tag="y")
                    nc.vector.tensor_copy(y_sb[:], ps[:])
                    yq = nc.sync if (2 * m + ph) % 2 == 0 else nc.scalar
                    yq.dma_start(
                        y_d[m * 128:(m + 1) * 128, ph * 384:(ph + 1) * 384],
                        y_sb[:])

            # ---- schedule ----
            # Emission order = dataflow order (tile deps derive from it);
            # the scheduler's greedy choice is steered by the bands above.
            qk_tile(0, halves=(0,))
            qk_tile(6, halves=(0,))
            qk_tile(0, halves=(1,))
            qk_tile(6, halves=(1,))
            qk_tile(1)
            qk_tile(7)
            for m in range(MT):
                v_tile(m)
            attention_nh(0, 0)
            for t in range(1, 6):
                if t < 5:
                    qk_tile(t + 1)
                    qk_tile(6 + t + 1)
                attention_nh(t, 0)
            for m in range(4):
                proj(m)
            for t in range(6):
                attention_nh(t, 1, last=(t == 5))
            for m in range(4, MT):
                proj(m)

    nc.compile()
    return nc


def _run(inputs, trace=False, trace_kwargs=None):
    global _cached_nc
    x = np.asarray(inputs["x"], dtype=np.float32)
    wqkv = np.ascontiguousarray(
        np.asarray(inputs["W_qkv"], dtype=np.float32)).astype(np.float16)
    wproj = np.ascontiguousarray(
        np.asarray(inputs["W_proj"], dtype=np.float32)).astype(np.float16)
    xT = np.ascontiguousarray(x.transpose(0, 2, 1)).astype(np.float16)

    if _cached_nc is None:
        _cached_nc = build_program()
    nc = _cached_nc

    in_maps = [{"xT": xT[b], "wqkv": wqkv, "wproj": wproj} for b in range(B)]
    kwargs = {}
    if trace:
        kwargs["trace"] = True
        if trace_kwargs:
            kwargs.update(trace_kwargs)
    try:
        res = run_bass_kernel_spmd(nc, in_maps, core_ids=list(range(B)), **kwargs)
    except Exception:
        # transient axon/PJRT hiccups happen; one retry
        res = run_bass_kernel_spmd(nc, in_maps, core_ids=list(range(B)), **kwargs)
    out = np.stack([r["y"] for r in res.results], axis=0)
    return out, res


def kernel(**inputs):
    out, _ = _run(inputs)
    return out
